# revision 5
# baseline (speedup 1.0000x reference)
"""LocallyConnected2d (64,64,32,32) x (1,64,64,32,32,9) -> (64,64,32,32) on 8 trn2 cores.

Strategy
--------
Spatial sharding over output rows: core i computes output rows [4i, 4i+4).

Per output location (x, y) the op is an independent GEMM:
    out[:, :, x, y] = patches(x,y) @ W(x,y).T + bias(:, x, y)
with contraction over (c, k) = 64*9 = 576, M = 64 out-channels, N = 64 batch.

Device scheme (8 matmuls per horizontal location pair A=(x,y), B=(x,y+1)):
  - x band in SBUF as [128, 13056]: partitions 0-63 hold channels (copy A,
    layout (h, w, b), b innermost), partitions 64-127 the same data shifted
    by +1 ROW (copy C), so a K=128 matmul contracts (c, kh) and (c, kh+1)
    at once (vertical tap stacking).
  - out partitions are M = [A out-ch | B out-ch] (horizontal M-pairing), so
    the big weight loads are 128 columns wide (fast-weight-load eligible):
      P1 (M128 K128) rhs cell (x, yA+1): A taps (0,1)(1,1), B taps (0,0)(1,0)
      P2 (M128 K128) rhs cell (x, yA+2): A (0,2)(1,2), B (0,1)(1,1)
      P3 (M64  K128) rhs cell (x, yA):   A (0,0)(1,0)        -> psum[0:64]
      P4 (M64  K128) rhs cell (x, yA+3): B (0,2)(1,2)        -> psum[64:128]
      P5 (M128 K64)  rhs cell (x+2,yA+1) lower: A (2,1), B (2,0)
      P6 (M128 K64)  rhs upper@(x+1,yA+2)=(x+2,yA+2): A (2,2), B (2,1)
      P7 (M64  K64)  rhs cell (x+2,yA) lower:  A (2,0)       -> psum[0:64]
      P8 (M64  K64)  rhs upper@(x+1,yA+3)=(x+2,yA+3): B (2,2)-> psum[64:128]
    Weight tile stays a dense [128, 576] block per pair (same HBM bytes).
  - weights host-prepacked to the exact [K, M] SBUF layout; all 8 tiles kept
    resident in SBUF so the weight stream never stalls on buffer recycling.
  - bias folded with one K=8 indicator matmul per PSUM bank.
  - x streamed in 3 row-chunks so the first matmuls start ~4us in; per tile
    the P1-P4 matmuls (rows x, x+1 only) are emitted before P5-P8 (row x+2).
  - output copied PSUM->SBUF as fp16 (half the output DMA); host casts back.

Compute dtype fp16 (fp32 accumulate in PSUM).
"""

import numpy as np

N_B, C, H, W_W, O = 64, 64, 32, 32, 64
KH = KW = 3
NCORES = 8
RPC = H // NCORES            # 4 output rows per core
BAND = RPC + 2               # 6 padded input rows per core
WP = W_W + 2                 # 34 padded width
ROWELEMS = WP * N_B          # 2176 elements per band row
XFREE = BAND * ROWELEMS      # 13056, layout (h, w, b) -- b innermost
UPPER_END = 5 * ROWELEMS     # 10880: shifted copy covers band rows 1..5
NPAIR_CORE = RPC * W_W // 2  # 64 location pairs per core
NTILE = 8                    # PSUM tiles per core (8 pairs each)
PAIR_COLS = 576              # weight cols per location pair
W_FREE = NPAIR_CORE * PAIR_COLS  # 36864

COMPUTE_NP = np.float16      # np.float16 | np.float32 | ml_dtypes.bfloat16
OUT_NP = np.float32          # device output dtype (host casts to fp32)

_CACHE = {}


def _mybir_dt(np_dt):
    import concourse.mybir as mybir
    import ml_dtypes

    if np_dt == np.float16:
        return mybir.dt.float16
    if np_dt == np.float32:
        return mybir.dt.float32
    if np_dt == ml_dtypes.bfloat16:
        return mybir.dt.bfloat16
    raise ValueError(np_dt)


def build_nc(compute_np=None):
    """Build the (single-program) Bass kernel; same NEFF runs on all 8 cores."""
    import concourse.bass as bass  # noqa: F401
    import concourse.mybir as mybir
    import concourse.tile as tile
    from concourse import bacc
    from contextlib import ExitStack

    cdt = _mybir_dt(compute_np or COMPUTE_NP)
    odt = _mybir_dt(OUT_NP)
    f32 = mybir.dt.float32

    nc = bacc.Bacc("TRN2", target_bir_lowering=False, debug=False)

    x_dram = nc.dram_tensor("xb", [64, XFREE], cdt, kind="ExternalInput")
    w_dram = nc.dram_tensor("wp", [128, W_FREE], cdt, kind="ExternalInput")
    b_dram = nc.dram_tensor("bp", [8, NTILE * 128], cdt, kind="ExternalInput")
    i_dram = nc.dram_tensor("ind", [8, 512], cdt, kind="ExternalInput")
    o_dram = nc.dram_tensor("out", [NTILE, 128, 512], odt, kind="ExternalOutput")

    with ExitStack() as ctx:
        tc = ctx.enter_context(tile.TileContext(nc))
        const = ctx.enter_context(tc.tile_pool(name="const", bufs=1))
        wpool = ctx.enter_context(tc.tile_pool(name="wpool", bufs=8))
        ppool = ctx.enter_context(tc.tile_pool(name="ppool", bufs=6, space="PSUM"))
        spool = ctx.enter_context(tc.tile_pool(name="spool", bufs=4))

        xsb = const.tile([128, XFREE], cdt)
        bias_sb = const.tile([8, NTILE * 128], cdt)
        ind_sb = const.tile([8, 512], cdt)

        nc.gpsimd.dma_start(bias_sb[:], b_dram.ap()[:, :])
        nc.gpsimd.dma_start(ind_sb[:], i_dram.ap()[:, :])

        # x free layout: f = (h*34 + w)*64 + b. Streamed in 3 row-pair chunks
        # on the sync queue; the +1-row copy (partitions 64-127) is built by
        # the idle vector engine as each chunk lands.
        R2 = 2 * ROWELEMS
        for ch in range(3):
            lo, hi = ch * R2, (ch + 1) * R2
            nc.sync.dma_start(xsb[0:64, lo:hi], x_dram.ap()[:, lo:hi])
            # upper[f] = lower[f + ROWELEMS]; chunk ch enables upper range
            # [max(0, lo-ROWELEMS) : hi-ROWELEMS)
            ulo = max(0, lo - ROWELEMS)
            uhi = hi - ROWELEMS
            nc.vector.tensor_copy(
                xsb[64:128, ulo:uhi], xsb[0:64, ulo + ROWELEMS : uhi + ROWELEMS]
            )

        x4 = xsb[:].rearrange("p (h w b) -> p h w b", h=BAND, w=WP)  # [128,6,34,64]

        for t in range(NTILE):
            wt = wpool.tile([128, 8 * PAIR_COLS], cdt)
            wbase = t * 8 * PAIR_COLS
            if t == 0:
                # split w0 into 4 chunks so pair-0 matmuls start early
                for q4 in range(4):
                    c0 = q4 * 2 * PAIR_COLS
                    c1 = (q4 + 1) * 2 * PAIR_COLS
                    nc.scalar.dma_start(
                        wt[:, c0:c1], w_dram.ap()[:, wbase + c0 : wbase + c1]
                    )
            else:
                weng = nc.scalar if t % 2 == 1 else nc.sync
                weng.dma_start(
                    wt[:], w_dram.ap()[:, wbase : wbase + 8 * PAIR_COLS]
                )
            ps = ppool.tile([128, 512], f32)
            xh = t // 2
            # P1-P4 first (need only band rows xh, xh+1), then P5-P8 (row xh+2)
            for jp in range(8):
                yA = 2 * ((t % 2) * 8 + jp)
                base = jp * PAIR_COLS
                oc = jp * 64
                mm = nc.tensor.matmul
                # P1/P2: M128 K128 shared-column pairs
                mm(ps[:, oc : oc + 64], wt[:, base : base + 128],
                   x4[:, xh, yA + 1, :],
                   start=(jp == 0), stop=False, skip_group_check=True)
                mm(ps[:, oc : oc + 64], wt[:, base + 128 : base + 256],
                   x4[:, xh, yA + 2, :],
                   start=False, stop=False, skip_group_check=True)
                # P3/P4: M64 K128 exclusive columns
                mm(ps[0:64, oc : oc + 64], wt[:, base + 256 : base + 320],
                   x4[:, xh, yA, :],
                   start=False, stop=False, skip_group_check=True)
                mm(ps[64:128, oc : oc + 64], wt[:, base + 320 : base + 384],
                   x4[:, xh, yA + 3, :],
                   start=False, stop=False, skip_group_check=True)
            for jp in range(8):
                yA = 2 * ((t % 2) * 8 + jp)
                base = jp * PAIR_COLS
                oc = jp * 64
                mm = nc.tensor.matmul
                # kh=2 row: 6 M64 K64 singles (A on partitions 0-63 via the
                # plain copy, B on 64-127 via the +1-row copy)
                for q in range(3):
                    cs = base + 384 + 64 * q
                    mm(ps[0:64, oc : oc + 64], wt[0:64, cs : cs + 64],
                       x4[0:64, xh + 2, yA + q, :],
                       start=False, stop=False, skip_group_check=True)
                    mm(ps[64:128, oc : oc + 64], wt[64:128, cs : cs + 64],
                       x4[64:128, xh + 1, yA + 1 + q, :],
                       start=False, stop=False, skip_group_check=True)
            # bias: psum[p, j*64+b] += bias[j, t*128+p] * ind[j, col]
            nc.tensor.matmul(
                ps[:, :],
                bias_sb[:, t * 128 : (t + 1) * 128],
                ind_sb[:, :],
                start=False,
                stop=True,
                skip_group_check=True,
            )
            stg = spool.tile([128, 512], odt)
            nc.vector.tensor_copy(stg[:], ps[:])
            oeng = nc.sync if t == NTILE - 1 else nc.gpsimd
            oeng.dma_start(o_dram.ap()[t], stg[:])

    nc.compile()
    return nc


def pack_inputs(x, weight, bias, compute_np=None):
    """Full fp32 inputs -> list of 8 per-core input dicts (device layouts)."""
    cnp = compute_np or COMPUTE_NP
    x = np.asarray(x)
    w5 = np.asarray(weight)[0]        # (o, c, x, y, k)
    b3 = np.asarray(bias)[0]          # (o, x, y)

    xp = np.pad(x, ((0, 0), (0, 0), (1, 1), (1, 1)))  # (b, c, 34, 34)

    ind = np.zeros((8, 512), dtype=cnp)
    for j in range(8):
        ind[j, j * 64 : (j + 1) * 64] = 1.0

    in_maps = []
    for i in range(NCORES):
        band = xp[:, :, RPC * i : RPC * i + BAND, :]          # (b, c, 6, 34)
        xb = np.ascontiguousarray(band.transpose(1, 2, 3, 0)) # (c, 6, 34, b)
        xb = xb.astype(cnp).reshape(64, XFREE)

        wc = w5[:, :, RPC * i : RPC * (i + 1), :, :]          # (o, c, 4, 32, 9)
        # A = even output cols, B = odd; index [o, c, xh, jr, k]
        A = wc[:, :, :, 0::2, :]
        B = wc[:, :, :, 1::2, :]
        # blocks[xh, jr, part, col]; part = khalf*64 + c, col layout per pair:
        # [P1(128) P2(128) P3(64) P4(64) P56(128) P78(64)]
        blk = np.empty((4, 16, 128, PAIR_COLS), dtype=np.float32)

        def put(colsl, khalf, src):  # src[o, c, xh, jr] -> blk[xh, jr, khalf*64+c, colsl]
            blk[:, :, khalf * 64 : khalf * 64 + 64, colsl] = src.transpose(2, 3, 1, 0)

        # P1: A taps 1,4 ; B taps 0,3
        put(slice(0, 64), 0, A[..., 1]);   put(slice(64, 128), 0, B[..., 0])
        put(slice(0, 64), 1, A[..., 4]);   put(slice(64, 128), 1, B[..., 3])
        # P2: A taps 2,5 ; B taps 1,4
        put(slice(128, 192), 0, A[..., 2]); put(slice(192, 256), 0, B[..., 1])
        put(slice(128, 192), 1, A[..., 5]); put(slice(192, 256), 1, B[..., 4])
        # P3: A taps 0,3
        put(slice(256, 320), 0, A[..., 0]); put(slice(256, 320), 1, A[..., 3])
        # P4: B taps 2,5
        put(slice(320, 384), 0, B[..., 2]); put(slice(320, 384), 1, B[..., 5])
        # kh=2 singles: col block 384+64q holds A tap 6+q (partitions 0-63)
        # and B tap 6+q (partitions 64-127)
        for q in range(3):
            put(slice(384 + 64 * q, 448 + 64 * q), 0, A[..., 6 + q])
            put(slice(384 + 64 * q, 448 + 64 * q), 1, B[..., 6 + q])

        # tiles: t = 2*xh + th, pair jp: jr = th*8 + jp
        # blk[xh, (th, jp), part, col] -> wp[part, (xh, th, jp, col)]
        b6 = blk.reshape(4, 2, 8, 128, PAIR_COLS)
        wp = b6.transpose(3, 0, 1, 2, 4).reshape(128, W_FREE).astype(cnp)

        bc = b3[:, RPC * i : RPC * (i + 1), :]                # (o, 4, 32)
        # bp[jp, t*128 + half*64 + o] = bc[o, xh, 2*(th*8+jp)+half]
        bcr = bc.reshape(64, 4, 2, 8, 2)                      # o xh th jp half
        bp = bcr.transpose(3, 1, 2, 4, 0).reshape(8, NTILE * 128).astype(cnp)

        in_maps.append(
            {
                "xb": np.ascontiguousarray(xb),
                "wp": np.ascontiguousarray(wp),
                "bp": np.ascontiguousarray(bp),
                "ind": ind,
            }
        )
    return in_maps


def unpack_output(core_outs):
    """8 per-core [NTILE,128,512] arrays -> full (64, 64, 32, 32) output."""
    arr = np.stack(core_outs)                     # (core, t, part, col)
    arr = arr.reshape(8, 4, 2, 2, 64, 8, 64)      # core xh th half o jp b
    out = arr.transpose(6, 4, 0, 1, 2, 5, 3)      # b o core xh th jp half
    return np.ascontiguousarray(
        out.reshape(64, 64, 32, 32), dtype=np.float32
    )


def run_on_device(in_maps, trace=False, compute_np=None, **kwargs):
    from concourse import bass_utils

    key = ("nc", np.dtype(compute_np or COMPUTE_NP).name)
    if key not in _CACHE:
        _CACHE[key] = build_nc(compute_np)
    nc = _CACHE[key]
    res = bass_utils.run_bass_kernel_spmd(
        nc, in_maps, core_ids=list(range(NCORES)), trace=trace, **kwargs
    )
    return res


def kernel(x, weight, bias):
    in_maps = pack_inputs(x, weight, bias)
    res = run_on_device(in_maps)
    return unpack_output([r["out"] for r in res.results])


# revision 6
# speedup vs baseline: 1.2837x; 1.2837x over previous
"""LocallyConnected2d (64,64,32,32) x (1,64,64,32,32,9) -> (64,64,32,32) on 8 trn2 cores.

Strategy
--------
Spatial sharding over output rows: core i computes output rows [4i, 4i+4).

Per output location (x, y) the op is an independent GEMM:
    out[:, :, x, y] = patches(x,y) @ W(x,y).T + bias(:, x, y)
with contraction over (c, k) = 64*9 = 576, M = 64 out-channels, N = 64 batch.

On this stack every matmul pays a fresh LDWEIGHTS whose cost is the number of
weight COLUMNS / 1.2 GHz (no fast-weight-load), so the kernel minimizes total
loaded weight columns: 10 matmuls per horizontal location pair (A, B), all
M=64, eight of them K=128 contracting TWO taps at once:

  x band lives in SBUF twice (both with layout (h, w, b), b innermost):
    region1 [128, 13056]: partitions 0-63 plain, 64-127 shifted +1 COLUMN
    region2 [128,  8704]: partitions 0-63 plain, 64-127 shifted +1 ROW
  For A=(x,y) (psum rows 0-63) and B=(x,y+1) (psum rows 64-127):
    A1/A2/A3: region1 cells (x+kh, y),   kh=0..2 -> taps (kh,0)+(kh,1), K=128
    A4:       region2 cell  (x,    y+2)          -> taps (0,2)+(1,2),   K=128
    A5:       region1 lower (x+2,  y+2)          -> tap  (2,2),         K=64
    B1/B2/B3: region1 cells (x+kh, y+1)          -> taps (kh,0)+(kh,1), K=128
    B4:       region2 cell  (x,    y+3)          -> taps (0,2)+(1,2),   K=128
    B5:       region1 UPPER (x+2,  y+2) (=(x+2,y+3)) -> tap (2,2),      K=64
  640 loaded columns per pair vs 768 for the naive schedule; the weight tile
  stays a dense [128, 576] block per pair (A5/B5 share columns across
  partition halves), so HBM weight traffic is unchanged.

Weights are host-prepacked to the exact [K, M] SBUF layout and streamed in 8
tiles of 8 pairs; bias is folded in with one K=8 indicator matmul per PSUM
bank; outputs are copied PSUM->SBUF as fp16 (host casts back to fp32).
x streams in 3 row-pair chunks so the first matmuls start early; per tile the
matmuls needing band rows x..x+1 are emitted before those needing row x+2.

Compute dtype fp16 (fp32 accumulate in PSUM).
"""

import numpy as np

N_B, C, H, W_W, O = 64, 64, 32, 32, 64
KH = KW = 3
NCORES = 8
RPC = H // NCORES            # 4 output rows per core
BAND = RPC + 2               # 6 padded input rows per core
WP = W_W + 2                 # 34 padded width
ROWELEMS = WP * N_B          # 2176 elements per band row
XFREE = BAND * ROWELEMS      # 13056, layout (h, w, b) -- b innermost
X2ROWS = 4                   # region2 holds band rows 0..3 (+1-row shifted up top)
X2FREE = X2ROWS * ROWELEMS   # 8704
NPAIR_CORE = RPC * W_W // 2  # 64 location pairs per core
NTILE = 8                    # PSUM tiles per core (8 pairs each)
PAIR_COLS = 576              # weight cols per location pair
W_FREE = NPAIR_CORE * PAIR_COLS  # 36864

COMPUTE_NP = np.float16      # np.float16 | np.float32 | ml_dtypes.bfloat16
OUT_NP = np.float16          # device output dtype (host casts to fp32)

_CACHE = {}


def _mybir_dt(np_dt):
    import concourse.mybir as mybir
    import ml_dtypes

    if np_dt == np.float16:
        return mybir.dt.float16
    if np_dt == np.float32:
        return mybir.dt.float32
    if np_dt == ml_dtypes.bfloat16:
        return mybir.dt.bfloat16
    raise ValueError(np_dt)


def build_nc(compute_np=None):
    """Build the (single-program) Bass kernel; same NEFF runs on all 8 cores."""
    import concourse.bass as bass  # noqa: F401
    import concourse.mybir as mybir
    import concourse.tile as tile
    from concourse import bacc
    from contextlib import ExitStack

    cdt = _mybir_dt(compute_np or COMPUTE_NP)
    odt = _mybir_dt(OUT_NP)
    f32 = mybir.dt.float32

    nc = bacc.Bacc("TRN2", target_bir_lowering=False, debug=False)

    x_dram = nc.dram_tensor("xb", [64, XFREE], cdt, kind="ExternalInput")
    w_dram = nc.dram_tensor("wp", [128, W_FREE], cdt, kind="ExternalInput")
    b_dram = nc.dram_tensor("bp", [8, NTILE * 128], cdt, kind="ExternalInput")
    i_dram = nc.dram_tensor("ind", [8, 512], cdt, kind="ExternalInput")
    o_dram = nc.dram_tensor("out", [NTILE, 128, 512], odt, kind="ExternalOutput")

    with ExitStack() as ctx:
        tc = ctx.enter_context(tile.TileContext(nc))
        const = ctx.enter_context(tc.tile_pool(name="const", bufs=1))
        wpool = ctx.enter_context(tc.tile_pool(name="wpool", bufs=4))
        ppool = ctx.enter_context(tc.tile_pool(name="ppool", bufs=6, space="PSUM"))
        spool = ctx.enter_context(tc.tile_pool(name="spool", bufs=4))

        x1 = const.tile([128, XFREE], cdt)   # [plain | +1 col]
        x2 = const.tile([128, X2FREE], cdt)  # [plain | +1 row]
        bias_sb = const.tile([8, NTILE * 128], cdt)
        ind_sb = const.tile([8, 512], cdt)

        nc.gpsimd.dma_start(bias_sb[:], b_dram.ap()[:, :])
        nc.gpsimd.dma_start(ind_sb[:], i_dram.ap()[:, :])

        # x free layout: f = (h*34 + w)*64 + b. Streamed in 3 row-pair chunks;
        # the vector engine builds the shifted copies as each chunk lands.
        R2 = 2 * ROWELEMS
        for ch in range(3):
            lo, hi = ch * R2, (ch + 1) * R2
            nc.sync.dma_start(x1[0:64, lo:hi], x_dram.ap()[:, lo:hi])
            # region1 upper: +1 column = +64 elements
            uhi = min(hi, XFREE) - 64
            ulo = max(0, lo - 64)
            nc.vector.tensor_copy(x1[64:128, ulo:uhi], x1[0:64, ulo + 64 : uhi + 64])
            # region2 lower: plain copy of band rows 0..3
            if lo < X2FREE:
                chi = min(hi, X2FREE)
                nc.vector.tensor_copy(x2[0:64, lo:chi], x1[0:64, lo:chi])
            # region2 upper: +1 row = +2176 elements; chunk ch enables
            # f in [max(0, lo-2176) : hi-2176)
            u2lo = max(0, lo - ROWELEMS)
            u2hi = min(hi - ROWELEMS, X2FREE)
            if u2hi > u2lo:
                nc.vector.tensor_copy(
                    x2[64:128, u2lo:u2hi],
                    x1[0:64, u2lo + ROWELEMS : u2hi + ROWELEMS],
                )

        x4a = x1[:].rearrange("p (h w b) -> p h w b", h=BAND, w=WP)   # [128,6,34,64]
        x4b = x2[:].rearrange("p (h w b) -> p h w b", h=X2ROWS, w=WP) # [128,4,34,64]

        for t in range(NTILE):
            wt = wpool.tile([128, 8 * PAIR_COLS], cdt)
            wbase = t * 8 * PAIR_COLS
            if t == 0:
                # split w0 so pair-0 matmuls start before all 8 pairs land
                half = 4 * PAIR_COLS
                nc.scalar.dma_start(wt[:, 0:half], w_dram.ap()[:, wbase : wbase + half])
                nc.scalar.dma_start(
                    wt[:, half : 8 * PAIR_COLS],
                    w_dram.ap()[:, wbase + half : wbase + 8 * PAIR_COLS],
                )
            else:
                weng = nc.scalar if t % 2 == 1 else nc.sync
                weng.dma_start(wt[:], w_dram.ap()[:, wbase : wbase + 8 * PAIR_COLS])
            ps = ppool.tile([128, 512], f32)
            xh = t // 2
            # group 1: needs band rows xh, xh+1 only
            for jp in range(8):
                yA = 2 * ((t % 2) * 8 + jp)
                base = jp * PAIR_COLS
                oc = jp * 64
                mm = nc.tensor.matmul
                st = jp == 0
                # A1/B1: region1 cells (xh, yA/yA+1): taps (0,0)+(0,1)
                mm(ps[0:64, oc : oc + 64], wt[:, base : base + 64],
                   x4a[:, xh, yA, :], start=st, stop=False, skip_group_check=True)
                mm(ps[64:128, oc : oc + 64], wt[:, base + 256 : base + 320],
                   x4a[:, xh, yA + 1, :], start=st, stop=False, skip_group_check=True)
                # A2/B2: region1 cells (xh+1, .): taps (1,0)+(1,1)
                mm(ps[0:64, oc : oc + 64], wt[:, base + 64 : base + 128],
                   x4a[:, xh + 1, yA, :], start=False, stop=False, skip_group_check=True)
                mm(ps[64:128, oc : oc + 64], wt[:, base + 320 : base + 384],
                   x4a[:, xh + 1, yA + 1, :], start=False, stop=False, skip_group_check=True)
                # A4/B4: region2 cells (xh, yA+2/yA+3): taps (0,2)+(1,2)
                mm(ps[0:64, oc : oc + 64], wt[:, base + 192 : base + 256],
                   x4b[:, xh, yA + 2, :], start=False, stop=False, skip_group_check=True)
                mm(ps[64:128, oc : oc + 64], wt[:, base + 448 : base + 512],
                   x4b[:, xh, yA + 3, :], start=False, stop=False, skip_group_check=True)
            # group 2: needs band row xh+2
            for jp in range(8):
                yA = 2 * ((t % 2) * 8 + jp)
                base = jp * PAIR_COLS
                oc = jp * 64
                mm = nc.tensor.matmul
                # A3/B3: region1 cells (xh+2, .): taps (2,0)+(2,1)
                mm(ps[0:64, oc : oc + 64], wt[:, base + 128 : base + 192],
                   x4a[:, xh + 2, yA, :], start=False, stop=False, skip_group_check=True)
                mm(ps[64:128, oc : oc + 64], wt[:, base + 384 : base + 448],
                   x4a[:, xh + 2, yA + 1, :], start=False, stop=False, skip_group_check=True)
                # A5/B5: tap (2,2) singles; B5 reads the +1-col copy so both
                # land on cell (xh+2, yA+2) of region1
                mm(ps[0:64, oc : oc + 64], wt[0:64, base + 512 : base + 576],
                   x4a[0:64, xh + 2, yA + 2, :], start=False, stop=False,
                   skip_group_check=True)
                mm(ps[64:128, oc : oc + 64], wt[64:128, base + 512 : base + 576],
                   x4a[64:128, xh + 2, yA + 2, :], start=False, stop=False,
                   skip_group_check=True)
            # bias: psum[p, j*64+b] += bias[j, t*128+p] * ind[j, col]
            nc.tensor.matmul(
                ps[:, :],
                bias_sb[:, t * 128 : (t + 1) * 128],
                ind_sb[:, :],
                start=False,
                stop=True,
                skip_group_check=True,
            )
            stg = spool.tile([128, 512], odt)
            nc.vector.tensor_copy(stg[:], ps[:])
            oeng = nc.sync if t == NTILE - 1 else nc.gpsimd
            oeng.dma_start(o_dram.ap()[t], stg[:])

    nc.compile()
    return nc


def pack_inputs(x, weight, bias, compute_np=None):
    """Full fp32 inputs -> list of 8 per-core input dicts (device layouts)."""
    cnp = compute_np or COMPUTE_NP
    x = np.asarray(x)
    w5 = np.asarray(weight)[0]        # (o, c, x, y, k)
    b3 = np.asarray(bias)[0]          # (o, x, y)

    xp = np.pad(x, ((0, 0), (0, 0), (1, 1), (1, 1)))  # (b, c, 34, 34)

    ind = np.zeros((8, 512), dtype=cnp)
    for j in range(8):
        ind[j, j * 64 : (j + 1) * 64] = 1.0

    in_maps = []
    for i in range(NCORES):
        band = xp[:, :, RPC * i : RPC * i + BAND, :]          # (b, c, 6, 34)
        xb = np.ascontiguousarray(band.transpose(1, 2, 3, 0)) # (c, 6, 34, b)
        xb = xb.astype(cnp).reshape(64, XFREE)

        wc = w5[:, :, RPC * i : RPC * (i + 1), :, :]          # (o, c, 4, 32, 9)
        # A = even output cols, B = odd; index [o, c, xh, jr, k], k = 3*kh+kw
        A = wc[:, :, :, 0::2, :]
        B = wc[:, :, :, 1::2, :]
        # blocks[xh, jr, part, col]; col layout per pair:
        # [A1 A2 A3 A4 | B1 B2 B3 B4 | S] with 64 cols each; K-halves are the
        # two stacked taps (or A/B for the shared singles block S).
        blk = np.empty((4, 16, 128, PAIR_COLS), dtype=np.float32)

        def put(colsl, khalf, src):  # src[o, c, xh, jr]
            blk[:, :, khalf * 64 : khalf * 64 + 64, colsl] = src.transpose(2, 3, 1, 0)

        for kh in range(3):           # A1-A3 / B1-B3: taps (kh,0)+(kh,1)
            put(slice(64 * kh, 64 * kh + 64), 0, A[..., 3 * kh])
            put(slice(64 * kh, 64 * kh + 64), 1, A[..., 3 * kh + 1])
            put(slice(256 + 64 * kh, 320 + 64 * kh), 0, B[..., 3 * kh])
            put(slice(256 + 64 * kh, 320 + 64 * kh), 1, B[..., 3 * kh + 1])
        # A4/B4: taps (0,2)+(1,2)
        put(slice(192, 256), 0, A[..., 2]); put(slice(192, 256), 1, A[..., 5])
        put(slice(448, 512), 0, B[..., 2]); put(slice(448, 512), 1, B[..., 5])
        # singles: tap (2,2); A on partitions 0-63, B on 64-127
        put(slice(512, 576), 0, A[..., 8]); put(slice(512, 576), 1, B[..., 8])

        # tiles: t = 2*xh + th, pair jp: jr = th*8 + jp
        b6 = blk.reshape(4, 2, 8, 128, PAIR_COLS)
        wp = b6.transpose(3, 0, 1, 2, 4).reshape(128, W_FREE).astype(cnp)

        bc = b3[:, RPC * i : RPC * (i + 1), :]                # (o, 4, 32)
        # bp[jp, t*128 + half*64 + o] = bc[o, xh, 2*(th*8+jp)+half]
        bcr = bc.reshape(64, 4, 2, 8, 2)                      # o xh th jp half
        bp = bcr.transpose(3, 1, 2, 4, 0).reshape(8, NTILE * 128).astype(cnp)

        in_maps.append(
            {
                "xb": np.ascontiguousarray(xb),
                "wp": np.ascontiguousarray(wp),
                "bp": np.ascontiguousarray(bp),
                "ind": ind,
            }
        )
    return in_maps


def unpack_output(core_outs):
    """8 per-core [NTILE,128,512] arrays -> full (64, 64, 32, 32) output."""
    arr = np.stack(core_outs)                     # (core, t, part, col)
    arr = arr.reshape(8, 4, 2, 2, 64, 8, 64)      # core xh th half o jp b
    out = arr.transpose(6, 4, 0, 1, 2, 5, 3)      # b o core xh th jp half
    return np.ascontiguousarray(
        out.reshape(64, 64, 32, 32), dtype=np.float32
    )


def run_on_device(in_maps, trace=False, compute_np=None, **kwargs):
    from concourse import bass_utils

    key = ("nc", np.dtype(compute_np or COMPUTE_NP).name)
    if key not in _CACHE:
        _CACHE[key] = build_nc(compute_np)
    nc = _CACHE[key]
    res = bass_utils.run_bass_kernel_spmd(
        nc, in_maps, core_ids=list(range(NCORES)), trace=trace, **kwargs
    )
    return res


def kernel(x, weight, bias):
    in_maps = pack_inputs(x, weight, bias)
    res = run_on_device(in_maps)
    return unpack_output([r["out"] for r in res.results])
